# revision 5
# baseline (speedup 1.0000x reference)
"""Trainium2 Bass kernel for nn_CrossAttention_78271484002687.

Reference computation (B=16, N=8192, D=256, DK=DV=64, NQ=256):
    k = x @ w_ks.T; v = x @ w_vs.T
    attn = (q @ k.T).T * scale          # (b, n, nq); scale irrelevant to argmax
    idx = argmax(attn, -1); hardattn = one_hot(idx)      # output 2
    out = (hardattn.T @ v) @ w_fc.T                      # output 1

Kernel strategy (8 NeuronCores, data-parallel over batch, 2 batches/core):
  - Fold q @ w_ks into qw (256x256) once on device; attn = x @ qw.T.
  - attn computed in fp16 hi/lo 3-term split (hi@hi + hi@lo + lo@hi) which is
    fp32-grade accurate but 1 cycle/row on the PE instead of fp32's 4.
  - hardattn = (attn >= rowmax) computed on the vector engine from PSUM.
  - out folded: accx[q,:] = sum_{n: idx(n)=q} x[n,:] via PE matmul with the
    one-hot mask (fp16), then out = accx @ M where M = w_vs.T @ w_fc.T
    (256x256, precomputed on device in fp32).
  - x streamed in 2 MiB chunks (2048 tokens); hardattn streamed out likewise.
"""

import numpy as np

B, N, D, DK, DV, NQ = 16, 8192, 256, 64, 64, 256
NCORES = 8
BPC = B // NCORES  # batches per core
P = 128
TPC = 16           # 128-token tiles per chunk
CHUNK = P * TPC    # 2048 tokens per DMA chunk
NCHUNK = N // CHUNK

_cache = {}


def _build():
    import concourse.bacc as bacc
    import concourse.mybir as mybir
    from concourse.tile import TileContext
    from concourse.masks import make_identity

    F32 = mybir.dt.float32
    F16 = mybir.dt.float16
    AX = mybir.AxisListType.X
    GE = mybir.AluOpType.is_ge

    nc = bacc.Bacc()
    x_t = nc.declare_dram_parameter("x", [BPC, N, D], F32, isOutput=False)
    q_t = nc.declare_dram_parameter("q", [NQ, DK], F32, isOutput=False)
    wks_t = nc.declare_dram_parameter("w_ks", [DK, D], F32, isOutput=False)
    wvs_t = nc.declare_dram_parameter("w_vs", [DV, D], F32, isOutput=False)
    wfc_t = nc.declare_dram_parameter("w_fc", [D, DV], F32, isOutput=False)
    out_t = nc.declare_dram_parameter("out", [BPC, NQ, D], F32, isOutput=True)
    hard_t = nc.declare_dram_parameter("hard", [BPC, N, NQ], F32, isOutput=True)

    with TileContext(nc) as tc:
        with tc.tile_pool(name="const", bufs=1) as cs:
            ident = cs.tile([P, P], F32)
            make_identity(nc, ident[:])

            # ---- one-time precompute (all fp32) ----
            qwt_hi = cs.tile([P, 2, NQ], F16)
            qwt_lo = cs.tile([P, 2, NQ], F16)
            m_sb = cs.tile([P, 2, D], F32)

            with tc.tile_pool(name="pre_ps", bufs=1, space="PSUM") as pps:
                q_in = cs.tile([P, 2, DK], F32)
                nc.sync.dma_start(
                    out=q_in[:], in_=q_t.ap().rearrange("(c p) k -> p c k", p=P)
                )
                wks_in = cs.tile([DK, D], F32)
                nc.sync.dma_start(out=wks_in[:], in_=wks_t[:])
                wvs_in = cs.tile([DV, D], F32)
                nc.sync.dma_start(out=wvs_in[:], in_=wvs_t[:])
                wfc_in = cs.tile([P, 2, DV], F32)
                nc.sync.dma_start(
                    out=wfc_in[:], in_=wfc_t.ap().rearrange("(c p) v -> p c v", p=P)
                )

                # qT (64, 256) and wfcT (64, 256) via PE transposes
                qT_ps = pps.tile([DK, 2, P], F32)
                wfcT_ps = pps.tile([DV, 2, P], F32)
                for c in range(2):
                    nc.tensor.transpose(
                        out=qT_ps[:, c, :], in_=q_in[:, c, :], identity=ident[:]
                    )
                    nc.tensor.transpose(
                        out=wfcT_ps[:, c, :], in_=wfc_in[:, c, :], identity=ident[:]
                    )
                qT_sb = cs.tile([DK, 2 * P], F32)
                nc.scalar.copy(out=qT_sb[:], in_=qT_ps[:].rearrange("k c p -> k (c p)"))
                wfcT_sb = cs.tile([DV, 2 * P], F32)
                nc.scalar.copy(
                    out=wfcT_sb[:], in_=wfcT_ps[:].rearrange("v c p -> v (c p)")
                )

                # qwT[d, q'] = sum_k w_ks[k, d] qT[k, q']   (2 d-chunks)
                # M[d, dm]  = sum_v w_vs[v, d] wfcT[v, dm]
                qwt_ps = pps.tile([P, 2, NQ], F32)
                m_ps = pps.tile([P, 2, D], F32)
                for c in range(2):
                    nc.tensor.matmul(
                        out=qwt_ps[:, c, :],
                        lhsT=wks_in[:, c * P : (c + 1) * P],
                        rhs=qT_sb[:],
                        start=True,
                        stop=True,
                    )
                    nc.tensor.matmul(
                        out=m_ps[:, c, :],
                        lhsT=wvs_in[:, c * P : (c + 1) * P],
                        rhs=wfcT_sb[:],
                        start=True,
                        stop=True,
                    )
                nc.scalar.copy(out=qwt_hi[:], in_=qwt_ps[:])
                nc.vector.tensor_sub(out=qwt_lo[:], in0=qwt_ps[:], in1=qwt_hi[:])
                nc.scalar.copy(out=m_sb[:], in_=m_ps[:])

            # ---- main loop ----
            with (
                tc.tile_pool(name="io", bufs=2) as io,
                tc.tile_pool(name="wk", bufs=3) as wk,
                tc.tile_pool(name="ep", bufs=2) as ep,
                tc.tile_pool(name="ps_xt", bufs=2, space="PSUM") as ps_xt,
                tc.tile_pool(name="ps_at", bufs=2, space="PSUM") as ps_at,
                tc.tile_pool(name="ps_ax", bufs=1, space="PSUM") as ps_ax,
                tc.tile_pool(name="ps_ep", bufs=1, space="PSUM") as ps_ep,
            ):
                for b in range(BPC):
                    # one PSUM bank per q-chunk: a start=True matmul marks its
                    # whole bank pending-zero, so temporally-interleaved
                    # accumulation groups must not share a bank
                    accx_ps0 = ps_ax.tile([P, D], F32, tag="accx0")
                    accx_ps1 = ps_ax.tile([P, D], F32, tag="accx1")
                    accx_ps = (accx_ps0, accx_ps1)
                    for c in range(NCHUNK):
                        x_in = io.tile([P, TPC, D], F32, tag="x_in")
                        nc.sync.dma_start(
                            out=x_in[:],
                            in_=x_t[b, c * CHUNK : (c + 1) * CHUNK, :].rearrange(
                                "(t p) d -> p t d", p=P
                            ),
                        )
                        ha_out = io.tile([P, TPC, NQ], F32, tag="ha_out")
                        for t in range(TPC):
                            first = c == 0 and t == 0
                            last = c == NCHUNK - 1 and t == TPC - 1
                            xT_ps = ps_xt.tile([P, 2, P], F32)
                            for dc in range(2):
                                nc.tensor.transpose(
                                    out=xT_ps[:, dc, :],
                                    in_=x_in[:, t, dc * P : (dc + 1) * P],
                                    identity=ident[:],
                                )
                            xT_hi = wk.tile([P, 2, P], F16, tag="xT_hi")
                            xT_lo = wk.tile([P, 2, P], F16, tag="xT_lo")
                            nc.scalar.copy(out=xT_hi[:], in_=xT_ps[:])
                            nc.vector.tensor_sub(
                                out=xT_lo[:], in0=xT_ps[:], in1=xT_hi[:]
                            )
                            x16 = wk.tile([P, D], F16, tag="x16")
                            nc.scalar.copy(out=x16[:], in_=x_in[:, t, :])

                            attn_ps = ps_at.tile([P, NQ], F32)
                            mm = 0
                            for dc in range(2):
                                for lhs, rhs in (
                                    (xT_hi, qwt_hi),
                                    (xT_hi, qwt_lo),
                                    (xT_lo, qwt_hi),
                                ):
                                    nc.tensor.matmul(
                                        out=attn_ps[:],
                                        lhsT=lhs[:, dc, :],
                                        rhs=rhs[:, dc, :],
                                        start=(mm == 0),
                                        stop=(mm == 5),
                                    )
                                    mm += 1

                            mx = wk.tile([P, 1], F32, tag="mx")
                            nc.vector.reduce_max(out=mx[:], in_=attn_ps[:], axis=AX)
                            nc.vector.tensor_scalar(
                                out=ha_out[:, t, :],
                                in0=attn_ps[:],
                                scalar1=mx[:, 0:1],
                                scalar2=None,
                                op0=GE,
                            )
                            hard16 = wk.tile([P, NQ], F16, tag="hard16")
                            nc.gpsimd.tensor_copy(out=hard16[:], in_=ha_out[:, t, :])

                            for qc in range(2):
                                nc.tensor.matmul(
                                    out=accx_ps[qc][:],
                                    lhsT=hard16[:, qc * P : (qc + 1) * P],
                                    rhs=x16[:],
                                    start=first,
                                    stop=last,
                                )
                        nc.sync.dma_start(
                            out=hard_t[b, c * CHUNK : (c + 1) * CHUNK, :].rearrange(
                                "(t p) q -> p t q", p=P
                            ),
                            in_=ha_out[:],
                        )

                    # ---- batch epilogue: out = accx @ M ----
                    accx_sb = ep.tile([P, 2, D], F32, tag="accx_sb")
                    nc.scalar.copy(out=accx_sb[:, 0, :], in_=accx_ps0[:])
                    nc.scalar.copy(out=accx_sb[:, 1, :], in_=accx_ps1[:])
                    accxT_ps = ps_ep.tile([P, 2, 2, P], F32)
                    for qc in range(2):
                        for dc in range(2):
                            nc.tensor.transpose(
                                out=accxT_ps[:, dc, qc, :],
                                in_=accx_sb[:, qc, dc * P : (dc + 1) * P],
                                identity=ident[:],
                            )
                    accxT_sb = ep.tile([P, 2, 2, P], F32, tag="accxT_sb")
                    nc.scalar.copy(out=accxT_sb[:], in_=accxT_ps[:])
                    out2_ps = ps_ep.tile([P, 2, D], F32)
                    for qc in range(2):
                        for dc in range(2):
                            nc.tensor.matmul(
                                out=out2_ps[:, qc, :],
                                lhsT=accxT_sb[:, dc, qc, :],
                                rhs=m_sb[:, dc, :],
                                start=(dc == 0),
                                stop=(dc == 1),
                            )
                    out2_sb = ep.tile([P, 2, D], F32, tag="out2_sb")
                    nc.scalar.copy(out=out2_sb[:], in_=out2_ps[:])
                    nc.sync.dma_start(
                        out=out_t[b].rearrange("(c p) d -> p c d", p=P),
                        in_=out2_sb[:],
                    )
    nc.finalize()
    return nc


def _get_nc():
    if "nc" not in _cache:
        _cache["nc"] = _build()
    return _cache["nc"]


def kernel(x, q, w_ks, w_vs, w_fc):
    from concourse.bass_utils import run_bass_kernel_spmd

    x = np.ascontiguousarray(np.asarray(x, dtype=np.float32))
    q = np.ascontiguousarray(np.asarray(q, dtype=np.float32))
    w_ks = np.ascontiguousarray(np.asarray(w_ks, dtype=np.float32))
    w_vs = np.ascontiguousarray(np.asarray(w_vs, dtype=np.float32))
    w_fc = np.ascontiguousarray(np.asarray(w_fc, dtype=np.float32))

    nc = _get_nc()
    in_maps = [
        {
            "x": x[i * BPC : (i + 1) * BPC],
            "q": q,
            "w_ks": w_ks,
            "w_vs": w_vs,
            "w_fc": w_fc,
        }
        for i in range(NCORES)
    ]
    res = run_bass_kernel_spmd(nc, in_maps, core_ids=list(range(NCORES)))
    _cache["last_result"] = res
    out = np.concatenate([r["out"] for r in res.results], axis=0)
    hard = np.concatenate([r["hard"] for r in res.results], axis=0)
    return out, hard
